# revision 31
# baseline (speedup 1.0000x reference)
"""DualAttention Trainium2 kernel: 8-core data-parallel over batch.

Each NeuronCore processes one batch element [1024, 512] end to end:
q/k/v projections, spatial+channel gated value, 8-head attention, output
projection.

Engine assignment (the cost model charges DMA transfers to the issuing
engine, so DMA capacity is an engine resource):
- PE runs only GEMMs, all bf16 at 1 cycle/row; there are no PE
  transposes at all.
- Pool (gpsimd) issues all casting DMAs: fp32 DRAM -> bf16 SBUF for
  inputs and weights (half the bytes of an f32 load).
- SP issues every 16-bit XBAR transpose DMA (inputs -> [d, token]
  layout, gated values -> [token, d], attention output -> [d, token])
  plus the small f32 loads and the output stores.
- ACT is reserved for the exp stream (the attention pacer) + the few
  sigmoids; DVE handles bias adds, relu via add+max, gating multiplies,
  and the softmax normalization.

Attention runs as one global stream over 64 (head, chunk) score tiles
with a cross-head PV lag: scores -> exp -> (lagged) PV, so the exp
stream starts ~13us in and never pauses at head boundaries. The PV
product uses the bf16 probability chunks as stationary operands
(stationary loads are free) against the 64-wide gated values, emitting
the attention output in [q_token, d] order at half the PE rows of the
[d, q] formulation; a ones column accumulates the softmax denominator
in a parallel PSUM bank, and normalization happens in the PSUM drain
(reciprocal + per-partition multiply on DVE). The value/gating chain,
q/k projection column chunks, and output-projection partials all run
as PE fillers inside the stream.
"""
import numpy as np

import concourse.bass as bass
import concourse.tile as tile
from concourse import bacc, mybir
from concourse.bass_utils import run_bass_kernel_spmd

B, N, D = 8, 1024, 512
H, DH, HID = 8, 64, 256
P = 128
K4 = D // P          # 4 d-chunks
T8 = N // P          # 8 token chunks
M2 = HID // P        # 2 hidden chunks
NCORES = 8
LAG = 10             # global PV lag, in (head, chunk) stream ticks
NH8 = H * T8         # 64 stream ticks

F32 = mybir.dt.float32
F32R = mybir.dt.float32r
BF16 = mybir.dt.bfloat16
AF = mybir.ActivationFunctionType
OP = mybir.AluOpType
AX = mybir.AxisListType

WEIGHT_NAMES = ["Wq", "Wk", "Wv", "Wo", "Ws1", "Ws2", "Wc1", "Wc2"]
BIAS_NAMES = ["bq", "bk", "bv", "bo", "bs1", "bs2", "bc1", "bc2"]

_CACHE = {}


def _build():
    nc = bacc.Bacc("TRN2", target_bir_lowering=False)

    query_h = nc.dram_tensor("query", [N, D], F32, kind="ExternalInput")
    key_h = nc.dram_tensor("key_in", [N, D], F32, kind="ExternalInput")
    value_h = nc.dram_tensor("value", [N, D], F32, kind="ExternalInput")
    wshape = {"Wq": [D, D], "Wk": [D, D], "Wv": [D, D], "Wo": [D, D],
              "Ws1": [D, HID], "Ws2": [HID, D],
              "Wc1": [D, HID], "Wc2": [HID, D]}
    w_h = {nm: nc.dram_tensor(nm, wshape[nm], F32, kind="ExternalInput")
           for nm in WEIGHT_NAMES}
    b_h = {
        nm: nc.dram_tensor(nm, [HID if nm in ("bs1", "bc1") else D], F32,
                           kind="ExternalInput")
        for nm in BIAS_NAMES
    }
    out_h = nc.dram_tensor("out", [N, D], F32, kind="ExternalOutput")

    with tile.TileContext(nc) as tc:
        with tc.tile_pool(name="const", bufs=1) as cpool, \
             tc.tile_pool(name="wrest", bufs=1) as wrest, \
             tc.tile_pool(name="big", bufs=1) as big, \
             tc.tile_pool(name="xst", bufs=6) as xst, \
             tc.tile_pool(name="osb", bufs=4) as osp:
            # PSUM: scores 2x2 banks, PV accum 1, denominators 1, GEMM
            # groups 2 (exactly 8 banks)
            psS = tc.alloc_tile_pool(name="psS", bufs=2, space="PSUM",
                                     side="right")
            psO = tc.alloc_tile_pool(name="psO", bufs=1, space="PSUM",
                                     side="left")
            psM = tc.alloc_tile_pool(name="psM", bufs=1, space="PSUM",
                                     side="left")
            psP = tc.alloc_tile_pool(name="psP", bufs=2, space="PSUM",
                                     side="left")
            ptp = tc.alloc_tile_pool(name="ptp", bufs=LAG + 2)
            ocp = tc.alloc_tile_pool(name="ocp", bufs=2)
            rcp = tc.alloc_tile_pool(name="rcp", bufs=2)
            accp = tc.alloc_tile_pool(name="accp", bufs=1)

            # transposed inputs ([d-chunk, token] columns), projections,
            # gated values, attention output - all bf16
            qx = big.tile([P, K4 * N], BF16, tag="qx")
            kx = big.tile([P, K4 * N], BF16, tag="kx")
            vT_in = big.tile([P, K4 * N], BF16, tag="vT_in")
            qT = big.tile([P, K4 * N], BF16, tag="qT")
            kT = big.tile([P, K4 * N], BF16, tag="kT")
            vvT = big.tile([P, K4 * N], BF16, tag="vvT")   # vv -> Vs -> Vd
            s1T = big.tile([P, M2 * N], BF16, tag="s1T")
            vdE = big.tile([P, T8 * H * DH], BF16, tag="vdE")
            outT = big.tile([P, K4 * N], BF16, tag="outT")
            extv = vdE[:].rearrange("p (t h c) -> p t h c", t=T8, h=H)

            ones_f = cpool.tile([P, 1], F32, tag="ones1f")
            nc.vector.memset(ones_f[:], 1.0)
            ones_bf = cpool.tile([P, 1], BF16, tag="ones1")
            nc.vector.tensor_copy(ones_bf[:], ones_f[:])

            def load_weight_bf(name, rows, cols):
                # Pool-issued casting DMA: fp32 DRAM -> bf16 SBUF, one issue
                nk = rows // P
                wt = wrest.tile([P, nk * cols], BF16, tag=name)
                nc.gpsimd.dma_start(
                    wt[:].rearrange("p (k c) -> p k c", k=nk),
                    w_h[name][:].rearrange("(k p) c -> p k c", p=P))
                return [wt[:, k * cols:(k + 1) * cols] for k in range(nk)]

            _mcol_tiles = {}

            def load_weight_mcols(name, ms):
                # column-sliced bf16 cast load of a [D, D] weight
                if name in _mcol_tiles:
                    wt = _mcol_tiles[name]
                else:
                    wt = wrest.tile([P, K4 * D], BF16, tag=name)
                    _mcol_tiles[name] = wt
                wv = wt[:].rearrange("p (k c) -> p k c", k=K4)
                for m in ms:
                    nc.gpsimd.dma_start(
                        wv[:, :, m * P:(m + 1) * P],
                        w_h[name][:, m * P:(m + 1) * P].rearrange(
                            "(k p) c -> p k c", p=P))
                return [wt[:, k * D:(k + 1) * D] for k in range(K4)]

            def load_bias(name, rows):
                nk = rows // P
                bt = cpool.tile([P, nk], F32, tag=name)
                nc.sync.dma_start(
                    bt[:], b_h[name][:].rearrange("(k p) -> p k", p=P))
                return [bt[:, k:k + 1] for k in range(nk)]

            def load_x_pair(src_dram, t8, dstT):
                # Pool casting DMA (2 token chunks) + 8 SP XBAR transposes
                st = xst.tile([P, 2 * D], BF16, tag="xst")
                nc.gpsimd.dma_start(
                    st[:].rearrange("p (c d) -> p c d", c=2),
                    src_dram[t8 * P:(t8 + 2) * P, :].rearrange(
                        "(c p) d -> p c d", p=P))
                for j in range(2):
                    for k in range(K4):
                        nc.sync.dma_start(
                            dstT[:, k * N + (t8 + j) * P:
                                 k * N + (t8 + j + 1) * P],
                            st[:, j * D + k * P:j * D + (k + 1) * P],
                            transpose=True)

            # ---- DMA kickoff: q/k first (the attention stream gates on
            # them), value right behind, then the gating weights
            for t8 in range(0, T8, 2):
                load_x_pair(query_h, t8, qx)
                load_x_pair(key_h, t8, kx)
            wq_t = load_weight_mcols("Wq", [0])
            wk_t = load_weight_mcols("Wk", [0])
            bq_t = load_bias("bq", D)
            bk_t = load_bias("bk", D)
            for t8 in range(0, T8, 2):
                load_x_pair(value_h, t8, vT_in)
            wv_t = load_weight_bf("Wv", D, D)
            bv_t = load_bias("bv", D)
            ws1_t = load_weight_bf("Ws1", D, HID)
            bs1_t = load_bias("bs1", HID)
            ws2_t = load_weight_bf("Ws2", HID, D)
            bs2_t = load_bias("bs2", D)
            # channel MLP weights stay f32 (1-wide matmuls, cost-free)
            wc1 = wrest.tile([P, K4 * HID], F32, tag="Wc1")
            nc.sync.dma_start(
                wc1[:].rearrange("p (k c) -> p k c", k=K4),
                w_h["Wc1"][:].rearrange("(k p) c -> p k c", p=P))
            wc1_t = [wc1[:, k * HID:(k + 1) * HID] for k in range(K4)]
            bc1_t = load_bias("bc1", HID)
            wc2 = wrest.tile([P, M2 * D], F32, tag="Wc2")
            nc.sync.dma_start(
                wc2[:].rearrange("p (k c) -> p k c", k=M2),
                w_h["Wc2"][:].rearrange("(k p) c -> p k c", p=P))
            wc2_t = [wc2[:, k * D:(k + 1) * D] for k in range(M2)]
            bc2_t = load_bias("bc2", D)

            # ---- projections (bf16 stationary x bf16 moving)
            def proj_m(w_tiles, bias_tiles, xsrc, OUT, m):
                for half in range(2):
                    ps = psP.tile([P, 512], F32, tag="psP")
                    for k in range(K4):
                        nc.tensor.matmul(
                            ps[:],
                            w_tiles[k][:, m * P:(m + 1) * P],
                            xsrc[:, k * N + half * 512:
                                 k * N + half * 512 + 512],
                            start=(k == 0), stop=(k == K4 - 1))
                    c0 = m * N + half * 512
                    nc.vector.tensor_scalar_add(
                        out=OUT[:, c0:c0 + 512], in0=ps[:],
                        scalar1=bias_tiles[m][:])

            def vv_group(m, half):
                ps = psP.tile([P, 512], F32, tag="psP")
                for k in range(K4):
                    nc.tensor.matmul(
                        ps[:], wv_t[k][:, m * P:(m + 1) * P],
                        vT_in[:, k * N + half * 512:k * N + half * 512 + 512],
                        start=(k == 0), stop=(k == K4 - 1))
                c0 = m * N + half * 512
                nc.vector.tensor_scalar_add(
                    out=vvT[:, c0:c0 + 512], in0=ps[:], scalar1=bv_t[m][:])

            def s1_group(m, half):
                # relu fused as (ps + b) max 0 on DVE - no ACT involvement
                ps = psP.tile([P, 512], F32, tag="psP")
                for k in range(K4):
                    nc.tensor.matmul(
                        ps[:], ws1_t[k][:, m * P:(m + 1) * P],
                        vvT[:, k * N + half * 512:k * N + half * 512 + 512],
                        start=(k == 0), stop=(k == K4 - 1))
                c0 = m * N + half * 512
                nc.vector.tensor_scalar(
                    out=s1T[:, c0:c0 + 512], in0=ps[:],
                    scalar1=bs1_t[m][:], scalar2=0.0,
                    op0=OP.add, op1=OP.max)

            def sw_group(m, half):
                ps = psP.tile([P, 512], F32, tag="psP")
                for k in range(M2):
                    nc.tensor.matmul(
                        ps[:], ws2_t[k][:, m * P:(m + 1) * P],
                        s1T[:, k * N + half * 512:k * N + half * 512 + 512],
                        start=(k == 0), stop=(k == M2 - 1))
                sw = xst.tile([P, 512], BF16, tag="swt", bufs=2)
                nc.scalar.activation(sw[:], ps[:], AF.Sigmoid,
                                     bias=bs2_t[m][:])
                sl = slice(m * N + half * 512, m * N + half * 512 + 512)
                nc.vector.tensor_tensor(out=vvT[:, sl], in0=vvT[:, sl],
                                        in1=sw[:], op=OP.mult)

            def vch_group(m, half):
                # channel gate folded into the weight (wv_c = cw x Wv), so
                # this streams the pristine vT_in again
                ps = psP.tile([P, 512], F32, tag="psP")
                for k in range(K4):
                    nc.tensor.matmul(
                        ps[:], wv_c[:, k * D + m * P:k * D + (m + 1) * P],
                        vT_in[:, k * N + half * 512:k * N + half * 512 + 512],
                        start=(k == 0), stop=(k == K4 - 1))
                sl = slice(m * N + half * 512, m * N + half * 512 + 512)
                nc.vector.scalar_tensor_tensor(
                    out=vvT[:, sl], in0=ps[:], scalar=bv_t[m][:],
                    in1=vvT[:, sl], op0=OP.add, op1=OP.add)

            def vd_tr(t8):
                # gated V -> [token, d] layout via SP XBAR transposes
                for k in range(K4):
                    nc.sync.dma_start(
                        vdE[:, t8 * D + k * P:t8 * D + (k + 1) * P],
                        vvT[:, k * N + t8 * P:k * N + (t8 + 1) * P],
                        transpose=True)

            def profile_mlp():
                # channel profile: mean over tokens -> 2-layer MLP -> cw
                profr = []
                for k in range(K4):
                    pr = cpool.tile([P, 1], F32, tag=f"prof{k}")
                    nc.vector.reduce_sum(pr[:], vT_in[:, k * N:(k + 1) * N],
                                         axis=AX.X)
                    prr = cpool.tile([P, 1], F32, tag=f"profr{k}")
                    nc.vector.tensor_scalar_mul(prr[:], pr[:], 1.0 / N)
                    profr.append(prr)
                c1r = []
                for m in range(M2):
                    ps = psP.tile([P, 512], F32, tag="psP")
                    for k in range(K4):
                        nc.tensor.matmul(
                            ps[:, 0:1], wc1_t[k][:, m * P:(m + 1) * P],
                            profr[k][:], start=(k == 0), stop=(k == K4 - 1))
                    cr = cpool.tile([P, 1], F32, tag=f"c1r{m}")
                    nc.scalar.activation(cr[:], ps[:, 0:1], AF.Relu,
                                         bias=bc1_t[m][:])
                    c1r.append(cr)
                for m in range(K4):
                    ps = psP.tile([P, 512], F32, tag="psP")
                    for k in range(M2):
                        nc.tensor.matmul(
                            ps[:, 0:1], wc2_t[k][:, m * P:(m + 1) * P],
                            c1r[k][:], start=(k == 0), stop=(k == M2 - 1))
                    cw = cpool.tile([P, 1], F32, tag=f"cw{m}")
                    nc.scalar.activation(cw[:], ps[:, 0:1], AF.Sigmoid,
                                         bias=bc2_t[m][:])
                    # fold the channel gate into a scaled copy of Wv (cw
                    # indexes d_in = the partition dim of Wv's chunk m)
                    nc.vector.tensor_scalar_mul(
                        out=wv_c[:, m * D:(m + 1) * D],
                        in0=wv_t[m][:], scalar1=cw[:])

            wv_c = wrest.tile([P, K4 * D], BF16, tag="Wvc")

            # ---- attention stream pieces
            def scores_chunk(h, k8):
                p0 = (h % 2) * DH
                cc = (h // 2) * N
                ps = psS.tile([P, N], F32, tag="psS")
                for half in range(2):
                    nc.tensor.matmul(
                        ps[:, half * 512:(half + 1) * 512],
                        kT[p0:p0 + DH, cc + k8 * P:cc + (k8 + 1) * P],
                        qT[p0:p0 + DH,
                           cc + half * 512:cc + (half + 1) * 512],
                        start=True, stop=True)
                pt = ptp.tile([P, N], BF16, tag="pt")
                nc.scalar.activation(pt[:], ps[:], AF.Exp, scale=0.125)
                return pt

            def pv_chunk(h, k8, accO, accD, pt):
                # out[q, d] += P[k, q].T @ V[k, d]: 64 rows per matmul, the
                # probability chunk rides as stationary; the ones column
                # accumulates the softmax denominator. Only the first group
                # opens with start=True - its bank-wide pending-zero mark
                # zero-fills the other 7 interleaved accumulation regions.
                for q8 in range(T8):
                    lhs = pt[:, q8 * P:(q8 + 1) * P]
                    nc.tensor.matmul(
                        accO[:, q8 * DH:(q8 + 1) * DH],
                        lhs, extv[:, k8, h, :],
                        start=(k8 == 0 and q8 == 0), stop=(k8 == T8 - 1),
                        skip_group_check=True)
                    nc.tensor.matmul(
                        accD[:, q8:q8 + 1],
                        lhs, ones_bf[:],
                        start=(k8 == 0 and q8 == 0), stop=(k8 == T8 - 1),
                        skip_group_check=True)

            def drain_head(h, accO, accD, ocat):
                # normalization fused into the PSUM drain: reciprocal of the
                # per-q denominators, then per-partition multiply into the
                # head-pair staging tile
                p0 = (h % 2) * DH
                recD = rcp.tile([P, T8], F32, tag="recD")
                nc.vector.reciprocal(recD[:], accD[:, 0:T8])
                for q8 in range(T8):
                    nc.vector.tensor_scalar_mul(
                        out=ocat[:, q8 * P + p0:q8 * P + p0 + DH],
                        in0=accO[:, q8 * DH:(q8 + 1) * DH],
                        scalar1=recD[:, q8:q8 + 1])

            def pair_transpose(m, ocat):
                # [q, 2-head d] -> outT chunk m via SP XBAR
                for q8 in range(T8):
                    nc.sync.dma_start(
                        outT[:, m * N + q8 * P:m * N + (q8 + 1) * P],
                        ocat[:, q8 * P:(q8 + 1) * P],
                        transpose=True)

            acc_t = []

            def final_partial(q8):
                ps = psP.tile([P, 512], F32, tag="psP")
                for k in range(K4 - 1):
                    nc.tensor.matmul(
                        ps[:],
                        outT[:, k * N + q8 * P:k * N + (q8 + 1) * P],
                        wo_t[k][:],
                        start=(k == 0), stop=(k == K4 - 2))
                at = accp.tile([P, D], F32, tag=f"acc{q8}")
                nc.vector.tensor_tensor(out=at[:], in0=ps[:], in1=boB[:],
                                        op=OP.add)
                acc_t.append(at)

            # ---- pre-stream: q/k projections for head pair 0
            proj_m(wq_t, bq_t, qx, qT, 0)
            proj_m(wk_t, bk_t, kx, kT, 0)
            load_weight_mcols("Wq", [1, 2, 3])
            load_weight_mcols("Wk", [1, 2, 3])
            wo_t = load_weight_bf("Wo", D, D)
            boB = cpool.tile([P, D], F32, tag="boB")
            nc.sync.dma_start(boB[:], b_h["bo"][None, :].to_broadcast((P, D)))

            # ---- filler schedule (PE work placed under the exp ceiling)
            fillers = {}

            def put(g, fn):
                fillers.setdefault(g, []).append(fn)

            # token-half 0 of the gating chain must reach vd_tr before the
            # first PV (g=LAG); half 1 before PV hits chunk 4 (g=LAG+4)
            put(3, lambda: [vv_group(m, 0) for m in (0, 1)])
            put(4, lambda: [vv_group(m, 0) for m in (2, 3)])
            put(4, profile_mlp)
            put(5, lambda: [s1_group(m, 0) for m in (0, 1)])
            put(6, lambda: [sw_group(m, 0) for m in (0, 1, 2, 3)])
            put(7, lambda: [vch_group(m, 0) for m in (0, 1, 2, 3)])
            put(8, lambda: [vd_tr(t8) for t8 in (0, 1, 2, 3)])
            put(8, lambda: [vv_group(m, 1) for m in (0, 1)])
            put(9, lambda: [vv_group(m, 1) for m in (2, 3)])
            put(10, lambda: [s1_group(m, 1) for m in (0, 1)])
            put(11, lambda: [sw_group(m, 1) for m in (0, 1, 2, 3)])
            put(12, lambda: [vch_group(m, 1) for m in (0, 1, 2, 3)])
            put(13, lambda: [vd_tr(t8) for t8 in (4, 5, 6, 7)])
            for mm in (1, 2, 3):
                put(16 * mm - 3, lambda m=mm: proj_m(wq_t, bq_t, qx, qT, m))
                put(16 * mm - 2, lambda m=mm: proj_m(wk_t, bk_t, kx, kT, m))
            for q8 in range(T8):
                put(59 + q8, lambda q=q8: final_partial(q))

            # ---- the unified scores/exp/PV stream
            pts = {}
            accO = accD = ocat = None
            ocats = {}
            for g in range(NH8 + LAG):
                if g < NH8:
                    h, k8 = g // T8, g % T8
                    pts[g] = scores_chunk(h, k8)
                gp = g - LAG
                if gp >= 0:
                    h, k8 = gp // T8, gp % T8
                    if k8 == 0:
                        accO = psO.tile([P, T8 * DH], F32, tag="acc")
                        accD = psM.tile([P, T8], F32, tag="den")
                        if h % 2 == 0:
                            ocat = ocp.tile([P, T8 * P], BF16, tag="ocat")
                            ocats[h // 2] = ocat
                    pv_chunk(h, k8, accO, accD, pts.pop(gp))
                    if k8 == T8 - 1:
                        drain_head(h, accO, accD, ocat)
                        if h % 2 == 1:
                            pair_transpose(h // 2, ocats.pop(h // 2))
                if g in fillers:
                    for fn in fillers[g]:
                        fn()

            # ---- tail: out = acc + outT_3.T @ Wo_3
            for q8 in range(T8):
                ps = psP.tile([P, D], F32, tag="psP")
                nc.tensor.matmul(
                    ps[:],
                    outT[:, 3 * N + q8 * P:3 * N + (q8 + 1) * P],
                    wo_t[3][:], start=True, stop=True)
                ob = osp.tile([P, D], F32, tag="osb")
                nc.vector.tensor_tensor(out=ob[:], in0=ps[:],
                                        in1=acc_t[q8][:], op=OP.add)
                nc.sync.dma_start(out_h[q8 * P:(q8 + 1) * P, :], ob[:])

            accp.release()
            rcp.release()
            ocp.release()
            ptp.release()
            psP.release()
            psM.release()
            psO.release()
            psS.release()

    nc.finalize()
    return nc


def get_nc():
    if "nc" not in _CACHE:
        _CACHE["nc"] = _build()
    return _CACHE["nc"]


def kernel(**inputs):
    if "key_in" not in inputs and "key" in inputs:
        inputs["key_in"] = inputs.pop("key")
    nc = get_nc()
    shared = {}
    for nm in WEIGHT_NAMES + BIAS_NAMES:
        shared[nm] = np.ascontiguousarray(np.asarray(inputs[nm], np.float32))
    in_maps = []
    for c in range(NCORES):
        m = dict(shared)
        m["query"] = np.ascontiguousarray(
            np.asarray(inputs["query"][c], np.float32))
        m["key_in"] = np.ascontiguousarray(
            np.asarray(inputs["key_in"][c], np.float32))
        m["value"] = np.ascontiguousarray(
            np.asarray(inputs["value"][c], np.float32))
        in_maps.append(m)
    res = run_bass_kernel_spmd(nc, in_maps, core_ids=list(range(NCORES)))
    return np.stack([res.results[c]["out"] for c in range(NCORES)], axis=0)
